# revision 12
# baseline (speedup 1.0000x reference)
"""Sparse (prefix-block + diagonal) masked attention on 8 TRN2 NeuronCores.

Problem: out[n,q,:] = softmax_s(mask(QK^T/8))[n,q,:] @ V[n] with
mask = (s < prefix_len[n]) | (s == q), N=8, S=2048, D=V=64, fp32.

Key ideas
---------
1. Only key columns s < prefix_len[n] plus the diagonal survive the mask, so
   the device computes unnormalized attention over the first
   ceil(p_n/128)*128 key columns only:
       A[v, q] = sum_{s<p} e(q.k_s) v_s,   Z[q] = sum_{s<p} e(q.k_s)
   with e(x) = exp(x/8)/8 (the extra 1/8, folded into the host-packed V,
   keeps A and Z inside fp16 range; it cancels in the host normalize).
   The diagonal term and out = (A + t v_q)/(Z + t) are host-side O(N*S*D).

2. Sharding: every core owns 256 query rows of EVERY batch element -> one
   SPMD program, perfectly balanced despite skewed prefix lengths.

3. Scores are computed TRANSPOSED (ST[s_tile, q] = K_tile^T . Q) so the
   e()'d tiles feed the PV matmul directly; Z comes from a ones-column in V.

4. PAIR=True packs TWO K=64 score tiles concurrently into PE row groups
   0-63 / 64-127 (halves score streaming time) and splits each K=128 PV
   matmul into two K=64 row-halves so no full-K matmul coexists with
   row-group ops.  PAIR=False is the safe classic layout (all operands in
   partitions 0-63 for scores, full-K PV).

5. exp splits across engines per group: ScalarE takes PSUM banks 0-1
   (exact EXP activation), VectorE takes bank 2 (Schraudolph bit-trick:
   one tensor_scalar writing fp16 bits through an int16 view, ~3% max rel
   err on 1/3 of scores).  PSUM->SBUF copies ride on VectorE; all matmul
   operands and the output are fp16.

6. DMA discipline: a handful of large contiguous transfers (qt first - the
   first matmul needs it) instead of per-group JIT streaming; dummy warm-up
   matmuls cover the initial fill so the PE clock-gate can open early.
"""

import numpy as np
from contextlib import ExitStack

import concourse.bacc as bacc
import concourse.tile as tile
import concourse.mybir as mybir
from concourse.bass_utils import run_bass_kernel_spmd

N, S, D, VD = 8, 2048, 64, 64
NCORES = 8
QPC = S // NCORES            # query rows per core per batch (256)
STS = 128                    # s-tile size
GROUP = 6                    # s-tiles per PSUM score group (3 banks)
SLOT = [0, 2, 4, 1, 3, 5]    # issue position in group -> 256-col slot
VW = VD + 1                  # V width with the ones column
NDUM = 10                    # PE warm-up dummy matmuls
PAIR = False                 # row-group score pairing (crashes some HW)

# exp(x/8) on VectorE as fp16 bits: i16 = round(x * SCH_A + SCH_B)
SCH_A = 1024.0 / np.log(2.0) * 0.125      # 184.664...
SCH_B = 15.0 * 1024.0 - 44.1              # -44.1 centers the +-3% ripple

LAST_RESULTS = None          # BassKernelResults of the most recent run (for test.py)

_program_cache = {}


def _plan(p):
    """Static plan derived from the prefix lengths (compile-time constants)."""
    p = [int(min(max(int(x), 0), S)) for x in p]
    T = [-(-x // STS) for x in p]                    # s-tiles per batch
    Ttot = sum(T)
    order = sorted(range(N), key=lambda n: -T[n])    # largest first
    seq = [(n, t) for n in order for t in range(T[n])]
    goff = {}
    g = 0
    for n in order:
        goff[n] = g
        g += T[n]
    npair = max((Ttot + 1) // 2, 1) if PAIR else max(Ttot, 1)
    return dict(p=p, T=T, Ttot=Ttot, w_kt=STS * npair, npair=npair, goff=goff,
                seq=seq, order=order)


# --------------------------------------------------------------------------
# host-side input packing
# --------------------------------------------------------------------------

KROWS = 128 if PAIR else 64


def _pack_shared(plan, K, V):
    """Packed K^T and V/8 with the ones/8 column, fp16."""
    p, T, Ttot = plan["p"], plan["T"], plan["Ttot"]
    ktp = np.zeros((KROWS, plan["w_kt"]), np.float32)
    vh = np.zeros((128, VW * max(Ttot, 1)), np.float32)
    g = 0
    for n in plan["order"]:
        for t in range(T[n]):
            lo, hi = STS * t, STS * (t + 1)
            nvalid = min(p[n], hi) - lo            # >=1 by construction
            blk = K[n, lo:hi, :].copy()
            blk[nvalid:, :] = 0.0
            if PAIR:
                r0, j = 64 * (g % 2), g // 2
            else:
                r0, j = 0, g
            ktp[r0:r0 + 64, STS * j:STS * (j + 1)] = blk.T
            vb = V[n, lo:hi, :] * 0.125            # 1/8 keeps A, Z in fp16
            vb[nvalid:, :] = 0.0
            vh[:, VW * g:VW * g + VD] = vb
            vh[:nvalid, VW * g + VD] = 0.125
            g += 1
    return ktp.astype(np.float16), vh.astype(np.float16)


def _pack_core(plan, Q, c):
    """Per-core input: transposed queries, fp16 (duplicated rows iff PAIR)."""
    qs = Q[:, QPC * c:QPC * (c + 1), :]                       # [N, 256, D]
    qt = qs.transpose(2, 0, 1).reshape(D, N * QPC).astype(np.float16)
    if PAIR:
        qt = np.concatenate([qt, qt], axis=0)
    return np.ascontiguousarray(qt)


# --------------------------------------------------------------------------
# device program
# --------------------------------------------------------------------------

def _build_program(key):
    plan = _plan(list(key))
    T, Ttot, seq, goff = plan["T"], plan["Ttot"], plan["seq"], plan["goff"]

    nc = bacc.Bacc("TRN2", target_bir_lowering=False, debug=False, num_devices=1)
    f32 = mybir.dt.float32
    f16 = mybir.dt.float16
    i16 = mybir.dt.int16
    EXP = mybir.ActivationFunctionType.Exp
    MUL = mybir.AluOpType.mult
    ADD = mybir.AluOpType.add

    ktp_d = nc.dram_tensor("ktp", [KROWS, plan["w_kt"]], f16, kind="ExternalInput").ap()
    qt_d = nc.dram_tensor("qt", [KROWS, S], f16, kind="ExternalInput").ap()
    vh_d = nc.dram_tensor("vh", [128, VW * max(Ttot, 1)], f16, kind="ExternalInput").ap()
    out_d = nc.dram_tensor("out", [VW, S], f16, kind="ExternalOutput").ap()

    with tile.TileContext(nc) as tc, ExitStack() as ctx:
        const = ctx.enter_context(tc.tile_pool(name="const", bufs=1))
        ktp = const.tile([KROWS, plan["w_kt"]], f16, tag="ktp")
        qt = const.tile([KROWS, S], f16, tag="qt")
        vh = const.tile([128, VW * max(Ttot, 1)], f16, tag="vh")
        out_sb = const.tile([VW, S], f16, tag="out_sb")

        if Ttot > 0:
            stp = ctx.enter_context(tc.tile_pool(name="stp", bufs=2, space="PSUM"))
            accp = ctx.enter_context(tc.tile_pool(name="accp", bufs=2, space="PSUM"))
            etp = ctx.enter_context(tc.tile_pool(name="etp", bufs=4))

            # ---- input streaming: few large transfers; qt first (the first
            # score matmul needs it), chunked ktp/vh so group 0 starts early
            pa = min(GROUP if PAIR else 2 * GROUP, plan["npair"])
            ta = min(2 * GROUP, Ttot)               # vh tiles for groups 0-1
            nc.sync.dma_start(qt[:], qt_d)
            nc.sync.dma_start(ktp[:, :STS * pa], ktp_d[:, :STS * pa])
            nc.scalar.dma_start(vh[:, :VW * ta], vh_d[:, :VW * ta])
            if plan["npair"] > pa:
                nc.sync.dma_start(ktp[:, STS * pa:], ktp_d[:, STS * pa:])
            if Ttot > ta:
                nc.scalar.dma_start(vh[:, VW * ta:VW * Ttot], vh_d[:, VW * ta:VW * Ttot])

            # ---- PE warm-up dummies (row group 0) into group 0's st tile
            dums = const.tile([64, 256], f16, tag="dums")
            nc.vector.memset(dums[:], 0.0)

            ngroups = (len(seq) + GROUP - 1) // GROUP
            outT = {}
            pv_cnt = [0] * N
            pending = []    # PV issued two groups late
            nz = sum(1 for x in T if x > 0)
            done_slots = [0]

            def _emit_pv(part, et):
                for i, (n, t) in enumerate(part):
                    if pv_cnt[n] == 0:
                        outT[n] = accp.tile([VW, 256], f32, tag="acc", name=f"outT{n}")
                    gi = int(goff[n]) + t
                    first = pv_cnt[n] == 0
                    last = pv_cnt[n] == T[n] - 1
                    sl = 256 * SLOT[i]
                    if PAIR:
                        # K=128 PV split into two concurrent K=64 row-halves
                        nc.tensor.matmul(
                            outT[n][:], vh[0:64, VW * gi:VW * gi + VW],
                            et[0:64, sl:sl + 256],
                            start=first, stop=False, tile_position=(0, 0),
                        )
                        nc.tensor.matmul(
                            outT[n][:], vh[64:128, VW * gi:VW * gi + VW],
                            et[64:128, sl:sl + 256],
                            start=False, stop=last, tile_position=(64, 0),
                        )
                    else:
                        nc.tensor.matmul(
                            outT[n][:], vh[:, VW * gi:VW * gi + VW],
                            et[:, sl:sl + 256], start=first, stop=last,
                        )
                    pv_cnt[n] += 1
                    if pv_cnt[n] == T[n]:
                        acc = outT.pop(n)
                        slot = plan["order"].index(n)
                        nc.vector.tensor_copy(
                            out_sb[:, QPC * slot:QPC * (slot + 1)], acc[:]
                        )
                        done_slots[0] += 1
                        half, penult = nz // 2, max(nz - 1, nz // 2)
                        if done_slots[0] == half and half > 0:
                            nc.sync.dma_start(
                                out_d[:, 0:QPC * half], out_sb[:, 0:QPC * half]
                            )
                        elif done_slots[0] == penult and penult > half:
                            nc.sync.dma_start(
                                out_d[:, QPC * half:QPC * penult],
                                out_sb[:, QPC * half:QPC * penult],
                            )
                        elif done_slots[0] == nz:
                            lo = QPC * penult
                            nc.sync.dma_start(
                                out_d[:, lo:QPC * nz], out_sb[:, lo:QPC * nz]
                            )

            for g in range(ngroups):
                part = seq[g * GROUP:(g + 1) * GROUP]
                st = stp.tile([128, 256 * GROUP], f32, tag="st")
                if g == 0:
                    for _ in range(NDUM):
                        nc.tensor.matmul(st[:64, 0:256], dums[:, :64], dums[:],
                                         start=True, stop=True)
                # scores; per bank exactly one accumulation group (start on
                # first write, stop on last)
                bank_writes = [0] * 3
                for i in range(len(part)):
                    bank_writes[SLOT[i] // 2] += 1
                bank_seen = [0] * 3
                for i, (n, t) in enumerate(part):
                    gi = int(goff[n]) + t
                    bank = SLOT[i] // 2
                    bank_seen[bank] += 1
                    start = bank_seen[bank] == 1
                    stop = bank_seen[bank] == bank_writes[bank]
                    sl = 256 * SLOT[i]
                    if PAIR:
                        r0, j = 64 * (gi % 2), gi // 2
                        nc.tensor.matmul(
                            st[:, sl:sl + 256],
                            ktp[r0:r0 + 64, STS * j:STS * (j + 1)],
                            qt[r0:r0 + 64, QPC * n:QPC * (n + 1)],
                            start=start, stop=stop, tile_position=(r0, 0),
                        )
                    else:
                        nc.tensor.matmul(
                            st[:, sl:sl + 256],
                            ktp[:, STS * gi:STS * (gi + 1)],
                            qt[:, QPC * n:QPC * (n + 1)],
                            start=start, stop=stop,
                        )
                et = etp.tile([128, 256 * GROUP], f16, tag="et")
                # exp only the written slots (partial last group is sparse),
                # as contiguous runs, split at the bank 0-1 / bank 2 boundary
                slots = sorted(SLOT[i] for i in range(len(part)))
                runs = []
                for sl in slots:
                    if runs and runs[-1][1] == sl:
                        runs[-1][1] = sl + 1
                    else:
                        runs.append([sl, sl + 1])
                for lo_s, hi_s in runs:
                    lo, hi = 256 * lo_s, 256 * hi_s
                    if lo < 1024:
                        h = min(hi, 1024)
                        nc.scalar.activation(et[:, lo:h], st[:, lo:h], EXP,
                                             scale=0.125)
                    if hi > 1024:
                        l = max(lo, 1024)
                        nc.vector.tensor_scalar(
                            et[:, l:hi].bitcast(i16), st[:, l:hi],
                            SCH_A, SCH_B, MUL, ADD,
                        )
                pending.append((part, et))
                if len(pending) > 2:
                    _emit_pv(*pending.pop(0))

            while pending:
                _emit_pv(*pending.pop(0))

        nempty = sum(1 for x in T if x == 0)
        if nempty:
            lo = QPC * (N - nempty)
            nc.vector.memset(out_sb[:, lo:QPC * N], 0.0)
            nc.sync.dma_start(out_d[:, lo:QPC * N], out_sb[:, lo:QPC * N])

    nc.compile()
    return nc, plan


# --------------------------------------------------------------------------
# entry point
# --------------------------------------------------------------------------

def kernel(queries_nqd, keys_nsd, values_nsv, prefix_len_n):
    global LAST_RESULTS
    Q = np.ascontiguousarray(np.asarray(queries_nqd, dtype=np.float32))
    K = np.ascontiguousarray(np.asarray(keys_nsd, dtype=np.float32))
    V = np.ascontiguousarray(np.asarray(values_nsv, dtype=np.float32))
    p = [int(x) for x in np.asarray(prefix_len_n)]

    key = tuple(min(max(x, 0), S) for x in p)
    if key not in _program_cache:
        _program_cache[key] = _build_program(key)
    nc, plan = _program_cache[key]

    ktp, vh = _pack_shared(plan, K, V)
    in_maps = [dict(ktp=ktp, qt=_pack_core(plan, Q, c), vh=vh) for c in range(NCORES)]

    res = run_bass_kernel_spmd(nc, in_maps, list(range(NCORES)))
    LAST_RESULTS = res

    # host-side gather: diagonal term + normalization (O(N*S*V) elementwise)
    # device A and Z carry an extra 1/8 (folded into vh); t matches it
    pa = np.asarray(plan["p"])
    t_nq = np.exp(np.einsum("nqd,nqd->nq", Q, K) * 0.125) * 0.125
    t_nq = np.where(np.arange(S)[None, :] >= pa[:, None], t_nq, 0.0).astype(np.float32)

    out = np.empty((N, S, VD), np.float32)
    for c in range(NCORES):
        oc = np.asarray(res.results[c]["out"], dtype=np.float32)   # [65, 2048]
        for slot, n in enumerate(plan["order"]):
            rows = slice(QPC * c, QPC * (c + 1))
            if plan["T"][n] == 0:
                out[n, rows, :] = V[n, rows, :]
                continue
            blk = oc[:, QPC * slot:QPC * (slot + 1)]           # [65, 256]
            A = blk[:VD, :].T                                  # [256, 64]
            Z = blk[VD, :]                                     # [256]
            t = t_nq[n, rows]
            out[n, rows, :] = (A + t[:, None] * V[n, rows, :]) / (Z + t)[:, None]
    return out


# revision 14
# speedup vs baseline: 1.0008x; 1.0008x over previous
"""Sparse (prefix-block + diagonal) masked attention on 8 TRN2 NeuronCores.

Problem: out[n,q,:] = softmax_s(mask(QK^T/8))[n,q,:] @ V[n] with
mask = (s < prefix_len[n]) | (s == q), N=8, S=2048, D=V=64, fp32.

Key ideas
---------
1. Only key columns s < prefix_len[n] plus the diagonal survive the mask, so
   the device computes unnormalized attention over the first
   ceil(p_n/128)*128 key columns only:
       A[v, q] = sum_{s<p} e(q.k_s) v_s,   Z[q] = sum_{s<p} e(q.k_s)
   with e(x) = exp(x/8)/8 (the extra 1/8, folded into the host-packed V,
   keeps A and Z inside fp16 range; it cancels in the host normalize).
   The diagonal term and out = (A + t v_q)/(Z + t) are host-side O(N*S*D).

2. Sharding: every core owns 256 query rows of EVERY batch element -> one
   SPMD program, perfectly balanced despite skewed prefix lengths.

3. Scores are computed TRANSPOSED (ST[s_tile, q] = K_tile^T . Q) so the
   e()'d tiles feed the PV matmul directly; Z comes from a ones-column in V.

4. PAIR=True packs TWO K=64 score tiles concurrently into PE row groups
   0-63 / 64-127 (halves score streaming time) and splits each K=128 PV
   matmul into two K=64 row-halves so no full-K matmul coexists with
   row-group ops.  PAIR=False is the safe classic layout (all operands in
   partitions 0-63 for scores, full-K PV).

5. exp splits across engines per group: ScalarE takes PSUM banks 0-1
   (exact EXP activation), VectorE takes bank 2 (Schraudolph bit-trick:
   one tensor_scalar writing fp16 bits through an int16 view, ~3% max rel
   err on 1/3 of scores).  PSUM->SBUF copies ride on VectorE; all matmul
   operands and the output are fp16.

6. DMA discipline: a handful of large contiguous transfers (qt first - the
   first matmul needs it) instead of per-group JIT streaming; dummy warm-up
   matmuls cover the initial fill so the PE clock-gate can open early.
"""

import numpy as np
from contextlib import ExitStack

import concourse.bacc as bacc
import concourse.tile as tile
import concourse.mybir as mybir
from concourse.bass_utils import run_bass_kernel_spmd

N, S, D, VD = 8, 2048, 64, 64
NCORES = 8
QPC = S // NCORES            # query rows per core per batch (256)
STS = 128                    # s-tile size
GROUP = 6                    # s-tiles per PSUM score group (3 banks)
SLOT = [0, 2, 4, 1, 3, 5]    # issue position in group -> 256-col slot
VW = VD + 1                  # V width with the ones column
NDUM = 14                   # PE warm-up dummy matmuls
PAIR = False                 # row-group score pairing (crashes some HW)

# exp(x/8) on VectorE as fp16 bits: i16 = round(x * SCH_A + SCH_B)
SCH_A = 1024.0 / np.log(2.0) * 0.125      # 184.664...
SCH_B = 15.0 * 1024.0 - 44.1              # -44.1 centers the +-3% ripple

LAST_RESULTS = None          # BassKernelResults of the most recent run (for test.py)

_program_cache = {}


def _plan(p):
    """Static plan derived from the prefix lengths (compile-time constants)."""
    p = [int(min(max(int(x), 0), S)) for x in p]
    T = [-(-x // STS) for x in p]                    # s-tiles per batch
    Ttot = sum(T)
    order = sorted(range(N), key=lambda n: -T[n])    # largest first
    seq = [(n, t) for n in order for t in range(T[n])]
    goff = {}
    g = 0
    for n in order:
        goff[n] = g
        g += T[n]
    npair = max((Ttot + 1) // 2, 1) if PAIR else max(Ttot, 1)
    return dict(p=p, T=T, Ttot=Ttot, w_kt=STS * npair, npair=npair, goff=goff,
                seq=seq, order=order)


# --------------------------------------------------------------------------
# host-side input packing
# --------------------------------------------------------------------------

KROWS = 128 if PAIR else 64


def _pack_shared(plan, K, V):
    """Packed K^T and V/8 with the ones/8 column, fp16."""
    p, T, Ttot = plan["p"], plan["T"], plan["Ttot"]
    ktp = np.zeros((KROWS, plan["w_kt"]), np.float32)
    vh = np.zeros((128, VW * max(Ttot, 1)), np.float32)
    g = 0
    for n in plan["order"]:
        for t in range(T[n]):
            lo, hi = STS * t, STS * (t + 1)
            nvalid = min(p[n], hi) - lo            # >=1 by construction
            blk = K[n, lo:hi, :].copy()
            blk[nvalid:, :] = 0.0
            if PAIR:
                r0, j = 64 * (g % 2), g // 2
            else:
                r0, j = 0, g
            ktp[r0:r0 + 64, STS * j:STS * (j + 1)] = blk.T
            vb = V[n, lo:hi, :] * 0.125            # 1/8 keeps A, Z in fp16
            vb[nvalid:, :] = 0.0
            vh[:, VW * g:VW * g + VD] = vb
            vh[:nvalid, VW * g + VD] = 0.125
            g += 1
    return ktp.astype(np.float16), vh.astype(np.float16)


def _pack_core(plan, Q, c):
    """Per-core input: transposed queries, fp16 (duplicated rows iff PAIR)."""
    qs = Q[:, QPC * c:QPC * (c + 1), :]                       # [N, 256, D]
    qt = qs.transpose(2, 0, 1).reshape(D, N * QPC).astype(np.float16)
    if PAIR:
        qt = np.concatenate([qt, qt], axis=0)
    return np.ascontiguousarray(qt)


# --------------------------------------------------------------------------
# device program
# --------------------------------------------------------------------------

def _build_program(key):
    plan = _plan(list(key))
    T, Ttot, seq, goff = plan["T"], plan["Ttot"], plan["seq"], plan["goff"]

    nc = bacc.Bacc("TRN2", target_bir_lowering=False, debug=False, num_devices=1)
    f32 = mybir.dt.float32
    f16 = mybir.dt.float16
    i16 = mybir.dt.int16
    EXP = mybir.ActivationFunctionType.Exp
    MUL = mybir.AluOpType.mult
    ADD = mybir.AluOpType.add

    ktp_d = nc.dram_tensor("ktp", [KROWS, plan["w_kt"]], f16, kind="ExternalInput").ap()
    qt_d = nc.dram_tensor("qt", [KROWS, S], f16, kind="ExternalInput").ap()
    vh_d = nc.dram_tensor("vh", [128, VW * max(Ttot, 1)], f16, kind="ExternalInput").ap()
    out_d = nc.dram_tensor("out", [VW, S], f16, kind="ExternalOutput").ap()

    with tile.TileContext(nc) as tc, ExitStack() as ctx:
        const = ctx.enter_context(tc.tile_pool(name="const", bufs=1))
        ktp = const.tile([KROWS, plan["w_kt"]], f16, tag="ktp")
        qt = const.tile([KROWS, S], f16, tag="qt")
        vh = const.tile([128, VW * max(Ttot, 1)], f16, tag="vh")
        out_sb = const.tile([VW, S], f16, tag="out_sb")

        if Ttot > 0:
            stp = ctx.enter_context(tc.tile_pool(name="stp", bufs=2, space="PSUM"))
            accp = ctx.enter_context(tc.tile_pool(name="accp", bufs=2, space="PSUM"))
            etp = ctx.enter_context(tc.tile_pool(name="etp", bufs=4))

            # ---- input streaming: few large transfers ordered by need-time
            # across the two HWDGE rings (sync + scalar), qt first
            sc = 2 if PAIR else 1                   # tiles per ktp column pair
            ka = min(2 * GROUP, Ttot) // sc         # ktp cols for groups 0-1
            kb = min(6 * GROUP, Ttot) // sc         # ... groups 2-5
            ta = min(2 * GROUP, Ttot)               # vh tiles for groups 0-1
            nc.sync.dma_start(qt[:], qt_d)
            nc.sync.dma_start(ktp[:, :STS * ka], ktp_d[:, :STS * ka])
            if kb > ka:
                nc.sync.dma_start(ktp[:, STS * ka:STS * kb], ktp_d[:, STS * ka:STS * kb])
            nc.scalar.dma_start(vh[:, :VW * ta], vh_d[:, :VW * ta])
            if plan["npair"] > kb:
                nc.scalar.dma_start(ktp[:, STS * kb:], ktp_d[:, STS * kb:])
            if Ttot > ta:
                nc.scalar.dma_start(vh[:, VW * ta:VW * Ttot], vh_d[:, VW * ta:VW * Ttot])

            # ---- PE warm-up dummies into group 0's st tile; +-1 checkerboard
            # data so the array actually toggles (an all-zero matmul may not
            # register as activity on the clock-gate monitor)
            dums = const.tile([64, 256], f16, tag="dums")
            nc.vector.memset(dums[:, 0::2], 1.0)
            nc.vector.memset(dums[:, 1::2], -1.0)

            ngroups = (len(seq) + GROUP - 1) // GROUP
            outT = {}
            pv_cnt = [0] * N
            pending = []    # PV issued two groups late
            nz = sum(1 for x in T if x > 0)
            done_slots = [0]

            def _emit_pv(part, et):
                for i, (n, t) in enumerate(part):
                    if pv_cnt[n] == 0:
                        outT[n] = accp.tile([VW, 256], f32, tag="acc", name=f"outT{n}")
                    gi = int(goff[n]) + t
                    first = pv_cnt[n] == 0
                    last = pv_cnt[n] == T[n] - 1
                    sl = 256 * SLOT[i]
                    if PAIR:
                        # K=128 PV split into two concurrent K=64 row-halves
                        nc.tensor.matmul(
                            outT[n][:], vh[0:64, VW * gi:VW * gi + VW],
                            et[0:64, sl:sl + 256],
                            start=first, stop=False, tile_position=(0, 0),
                        )
                        nc.tensor.matmul(
                            outT[n][:], vh[64:128, VW * gi:VW * gi + VW],
                            et[64:128, sl:sl + 256],
                            start=False, stop=last, tile_position=(64, 0),
                        )
                    else:
                        nc.tensor.matmul(
                            outT[n][:], vh[:, VW * gi:VW * gi + VW],
                            et[:, sl:sl + 256], start=first, stop=last,
                        )
                    pv_cnt[n] += 1
                    if pv_cnt[n] == T[n]:
                        acc = outT.pop(n)
                        slot = plan["order"].index(n)
                        nc.vector.tensor_copy(
                            out_sb[:, QPC * slot:QPC * (slot + 1)], acc[:]
                        )
                        done_slots[0] += 1
                        half, penult = nz // 2, max(nz - 1, nz // 2)
                        if done_slots[0] == half and half > 0:
                            nc.sync.dma_start(
                                out_d[:, 0:QPC * half], out_sb[:, 0:QPC * half]
                            )
                        elif done_slots[0] == penult and penult > half:
                            nc.sync.dma_start(
                                out_d[:, QPC * half:QPC * penult],
                                out_sb[:, QPC * half:QPC * penult],
                            )
                        elif done_slots[0] == nz:
                            lo = QPC * penult
                            nc.sync.dma_start(
                                out_d[:, lo:QPC * nz], out_sb[:, lo:QPC * nz]
                            )

            for g in range(ngroups):
                part = seq[g * GROUP:(g + 1) * GROUP]
                st = stp.tile([128, 256 * GROUP], f32, tag="st")
                if g == 0:
                    for _ in range(NDUM):
                        nc.tensor.matmul(st[:64, 0:256], dums[:, :64], dums[:],
                                         start=True, stop=True)
                # scores; per bank exactly one accumulation group (start on
                # first write, stop on last)
                bank_writes = [0] * 3
                for i in range(len(part)):
                    bank_writes[SLOT[i] // 2] += 1
                bank_seen = [0] * 3
                for i, (n, t) in enumerate(part):
                    gi = int(goff[n]) + t
                    bank = SLOT[i] // 2
                    bank_seen[bank] += 1
                    start = bank_seen[bank] == 1
                    stop = bank_seen[bank] == bank_writes[bank]
                    sl = 256 * SLOT[i]
                    if PAIR:
                        r0, j = 64 * (gi % 2), gi // 2
                        nc.tensor.matmul(
                            st[:, sl:sl + 256],
                            ktp[r0:r0 + 64, STS * j:STS * (j + 1)],
                            qt[r0:r0 + 64, QPC * n:QPC * (n + 1)],
                            start=start, stop=stop, tile_position=(r0, 0),
                        )
                    else:
                        nc.tensor.matmul(
                            st[:, sl:sl + 256],
                            ktp[:, STS * gi:STS * (gi + 1)],
                            qt[:, QPC * n:QPC * (n + 1)],
                            start=start, stop=stop,
                        )
                et = etp.tile([128, 256 * GROUP], f16, tag="et")
                # exp only the written slots (partial last group is sparse),
                # as contiguous runs, split at the bank 0-1 / bank 2 boundary
                slots = sorted(SLOT[i] for i in range(len(part)))
                runs = []
                for sl in slots:
                    if runs and runs[-1][1] == sl:
                        runs[-1][1] = sl + 1
                    else:
                        runs.append([sl, sl + 1])
                for lo_s, hi_s in runs:
                    lo, hi = 256 * lo_s, 256 * hi_s
                    if lo < 1024:
                        h = min(hi, 1024)
                        nc.scalar.activation(et[:, lo:h], st[:, lo:h], EXP,
                                             scale=0.125)
                    if hi > 1024:
                        l = max(lo, 1024)
                        nc.vector.tensor_scalar(
                            et[:, l:hi].bitcast(i16), st[:, l:hi],
                            SCH_A, SCH_B, MUL, ADD,
                        )
                pending.append((part, et))
                if len(pending) > 2:
                    _emit_pv(*pending.pop(0))

            while pending:
                _emit_pv(*pending.pop(0))

        nempty = sum(1 for x in T if x == 0)
        if nempty:
            lo = QPC * (N - nempty)
            nc.vector.memset(out_sb[:, lo:QPC * N], 0.0)
            nc.sync.dma_start(out_d[:, lo:QPC * N], out_sb[:, lo:QPC * N])

    nc.compile()
    return nc, plan


# --------------------------------------------------------------------------
# entry point
# --------------------------------------------------------------------------

def kernel(queries_nqd, keys_nsd, values_nsv, prefix_len_n):
    global LAST_RESULTS
    Q = np.ascontiguousarray(np.asarray(queries_nqd, dtype=np.float32))
    K = np.ascontiguousarray(np.asarray(keys_nsd, dtype=np.float32))
    V = np.ascontiguousarray(np.asarray(values_nsv, dtype=np.float32))
    p = [int(x) for x in np.asarray(prefix_len_n)]

    key = tuple(min(max(x, 0), S) for x in p)
    if key not in _program_cache:
        _program_cache[key] = _build_program(key)
    nc, plan = _program_cache[key]

    ktp, vh = _pack_shared(plan, K, V)
    in_maps = [dict(ktp=ktp, qt=_pack_core(plan, Q, c), vh=vh) for c in range(NCORES)]

    res = run_bass_kernel_spmd(nc, in_maps, list(range(NCORES)))
    LAST_RESULTS = res

    # host-side gather: diagonal term + normalization (O(N*S*V) elementwise)
    # device A and Z carry an extra 1/8 (folded into vh); t matches it
    pa = np.asarray(plan["p"])
    t_nq = np.exp(np.einsum("nqd,nqd->nq", Q, K) * 0.125) * 0.125
    t_nq = np.where(np.arange(S)[None, :] >= pa[:, None], t_nq, 0.0).astype(np.float32)

    out = np.empty((N, S, VD), np.float32)
    for c in range(NCORES):
        oc = np.asarray(res.results[c]["out"], dtype=np.float32)   # [65, 2048]
        for slot, n in enumerate(plan["order"]):
            rows = slice(QPC * c, QPC * (c + 1))
            if plan["T"][n] == 0:
                out[n, rows, :] = V[n, rows, :]
                continue
            blk = oc[:, QPC * slot:QPC * (slot + 1)]           # [65, 256]
            A = blk[:VD, :].T                                  # [256, 64]
            Z = blk[VD, :]                                     # [256]
            t = t_nq[n, rows]
            out[n, rows, :] = (A + t[:, None] * V[n, rows, :]) / (Z + t)[:, None]
    return out


# revision 16
# speedup vs baseline: 1.2081x; 1.2072x over previous
"""Sparse (prefix-block + diagonal) masked attention on 8 TRN2 NeuronCores.

Problem: out[n,q,:] = softmax_s(mask(QK^T/8))[n,q,:] @ V[n] with
mask = (s < prefix_len[n]) | (s == q), N=8, S=2048, D=V=64, fp32.

Key ideas
---------
1. Only key columns s < prefix_len[n] plus the diagonal survive the mask, so
   the device computes unnormalized attention over the first
   ceil(p_n/128)*128 key columns only:
       A[q, v] = sum_{s<p} e(q.k_s) v_s,   Z[q] = sum_{s<p} e(q.k_s)
   with e(x) = exp(x/8)/8 (the extra 1/8, folded into the host-packed V,
   keeps A and Z inside fp16 range; it cancels in the host normalize).
   The diagonal term and out = (A + t v_q)/(Z + t) are host-side O(N*S*D).

2. Sharding: every core owns 256 query rows of EVERY batch element -> one
   SPMD program, perfectly balanced despite skewed prefix lengths.

3. Scores are computed TRANSPOSED (ST[s_tile, q] = K_tile^T . Q, 256 moving
   q columns per s-tile).  The e()'d tile then feeds the PV matmul as the
   STATIONARY operand (two 128-wide q-halves), with V (+ a 1/8-ones column
   for Z) as the 65-column moving operand: PV streams 2x65 columns per
   s-tile instead of 256.  PE streaming cycles are the binding cost (the
   chip's power arbiter usually holds the PE at the cold 1.2 GHz clock).

4. Both q-halves of a batch accumulate into ONE per-batch PSUM tile
   [128, 130] in a single accumulation group (start on the very first
   half-matmul: its bank-wide has_written clear covers both halves; stop on
   the very last).

5. exp splits across engines per group: ScalarE takes PSUM banks 0-1
   (exact EXP activation), VectorE takes bank 2 (Schraudolph bit-trick:
   one tensor_scalar writing fp16 bits through an int16 view, ~3% max rel
   err on 1/3 of scores).  PSUM->SBUF copies ride on VectorE; all matmul
   operands and the output are fp16.

6. DMA discipline: all input transfers ride ONE HWDGE ring in strict
   need-order (qt, then interleaved ktp/vh chunks) so early chunks get the
   full 16-engine bandwidth; dummy warm-up matmuls (+-1 checkerboard) cover
   the initial fill.
"""

import numpy as np
from contextlib import ExitStack

import concourse.bacc as bacc
import concourse.tile as tile
import concourse.mybir as mybir
from concourse.bass_utils import run_bass_kernel_spmd

N, S, D, VD = 8, 2048, 64, 64
NCORES = 8
QPC = S // NCORES            # query rows per core per batch (256)
STS = 128                    # s-tile size
GROUP = 6                    # s-tiles per PSUM score group (3 banks)
SLOT = [0, 2, 4, 1, 3, 5]    # issue position in group -> 256-col slot
VW = VD + 1                  # V width with the ones column
OW = 2 * VW                  # per-batch output width (two q-halves)
NDUM = 14                    # PE warm-up dummy matmuls

# exp(x/8) on VectorE as fp16 bits: i16 = round(x * SCH_A + SCH_B)
SCH_A = 1024.0 / np.log(2.0) * 0.125      # 184.664...
SCH_B = 15.0 * 1024.0 - 44.1              # -44.1 centers the +-3% ripple

LAST_RESULTS = None          # BassKernelResults of the most recent run (for test.py)

_program_cache = {}


def _plan(p):
    """Static plan derived from the prefix lengths (compile-time constants)."""
    p = [int(min(max(int(x), 0), S)) for x in p]
    T = [-(-x // STS) for x in p]                    # s-tiles per batch
    Ttot = sum(T)
    order = sorted(range(N), key=lambda n: -T[n])    # largest first
    seq = [(n, t) for n in order for t in range(T[n])]
    goff = {}
    g = 0
    for n in order:
        goff[n] = g
        g += T[n]
    return dict(p=p, T=T, Ttot=Ttot, w_kt=STS * max(Ttot, 1), goff=goff,
                seq=seq, order=order)


# --------------------------------------------------------------------------
# host-side input packing
# --------------------------------------------------------------------------

def _pack_shared(plan, K, V):
    """Packed K^T and V/8 with the ones/8 column, fp16."""
    p, T, Ttot = plan["p"], plan["T"], plan["Ttot"]
    ktp = np.zeros((64, plan["w_kt"]), np.float32)
    vh = np.zeros((128, VW * max(Ttot, 1)), np.float32)
    g = 0
    for n in plan["order"]:
        for t in range(T[n]):
            lo, hi = STS * t, STS * (t + 1)
            nvalid = min(p[n], hi) - lo            # >=1 by construction
            blk = K[n, lo:hi, :].copy()
            blk[nvalid:, :] = 0.0
            ktp[:, STS * g:STS * (g + 1)] = blk.T
            vb = V[n, lo:hi, :] * 0.125            # 1/8 keeps A, Z in fp16
            vb[nvalid:, :] = 0.0
            vh[:, VW * g:VW * g + VD] = vb
            vh[:nvalid, VW * g + VD] = 0.125
            g += 1
    return ktp.astype(np.float16), vh.astype(np.float16)


def _pack_core(plan, Q, c):
    """Per-core input: transposed queries [64, 2048], fp16."""
    qs = Q[:, QPC * c:QPC * (c + 1), :]                       # [N, 256, D]
    qt = qs.transpose(2, 0, 1).reshape(D, N * QPC).astype(np.float16)
    return np.ascontiguousarray(qt)


# --------------------------------------------------------------------------
# device program
# --------------------------------------------------------------------------

def _build_program(key):
    plan = _plan(list(key))
    T, Ttot, seq, goff = plan["T"], plan["Ttot"], plan["seq"], plan["goff"]

    nc = bacc.Bacc("TRN2", target_bir_lowering=False, debug=False, num_devices=1)
    f32 = mybir.dt.float32
    f16 = mybir.dt.float16
    i16 = mybir.dt.int16
    EXP = mybir.ActivationFunctionType.Exp
    MUL = mybir.AluOpType.mult
    ADD = mybir.AluOpType.add

    ktp_d = nc.dram_tensor("ktp", [64, plan["w_kt"]], f16, kind="ExternalInput").ap()
    qt_d = nc.dram_tensor("qt", [64, S], f16, kind="ExternalInput").ap()
    vh_d = nc.dram_tensor("vh", [128, VW * max(Ttot, 1)], f16, kind="ExternalInput").ap()
    out_d = nc.dram_tensor("out", [128, OW * N], f16, kind="ExternalOutput").ap()

    with tile.TileContext(nc) as tc, ExitStack() as ctx:
        const = ctx.enter_context(tc.tile_pool(name="const", bufs=1))
        ktp = const.tile([64, plan["w_kt"]], f16, tag="ktp")
        qt = const.tile([64, S], f16, tag="qt")
        vh = const.tile([128, VW * max(Ttot, 1)], f16, tag="vh")
        out_sb = const.tile([128, OW * N], f16, tag="out_sb")

        if Ttot > 0:
            stp = ctx.enter_context(tc.tile_pool(name="stp", bufs=2, space="PSUM"))
            accp = ctx.enter_context(tc.tile_pool(name="accp", bufs=2, space="PSUM"))
            etp = ctx.enter_context(tc.tile_pool(name="etp", bufs=4))

            # ---- input streaming: ONE ring, strict need-order FIFO
            ka = min(2 * GROUP, Ttot)
            kb = min(6 * GROUP, Ttot)
            nc.sync.dma_start(qt[:], qt_d)
            nc.sync.dma_start(ktp[:, :STS * ka], ktp_d[:, :STS * ka])
            nc.sync.dma_start(vh[:, :VW * ka], vh_d[:, :VW * ka])
            if kb > ka:
                nc.sync.dma_start(ktp[:, STS * ka:STS * kb], ktp_d[:, STS * ka:STS * kb])
                nc.sync.dma_start(vh[:, VW * ka:VW * kb], vh_d[:, VW * ka:VW * kb])
            if Ttot > kb:
                nc.sync.dma_start(ktp[:, STS * kb:STS * Ttot], ktp_d[:, STS * kb:STS * Ttot])
                nc.sync.dma_start(vh[:, VW * kb:VW * Ttot], vh_d[:, VW * kb:VW * Ttot])

            # ---- PE warm-up dummies into group 0's st tile; +-1 checkerboard
            # so the array actually toggles on the activity monitor
            dums = const.tile([64, 256], f16, tag="dums")
            nc.vector.memset(dums[:, 0::2], 1.0)
            nc.vector.memset(dums[:, 1::2], -1.0)

            ngroups = (len(seq) + GROUP - 1) // GROUP
            outT = {}
            pv_cnt = [0] * N
            pending = []    # PV issued two groups late
            nz = sum(1 for x in T if x > 0)
            done_slots = [0]

            def _emit_pv(part, et):
                for i, (n, t) in enumerate(part):
                    if pv_cnt[n] == 0:
                        outT[n] = accp.tile([128, OW], f32, tag="acc", name=f"outT{n}")
                    gi = int(goff[n]) + t
                    first = pv_cnt[n] == 0
                    last = pv_cnt[n] == T[n] - 1
                    sl = 256 * SLOT[i]
                    # et (stationary, q-half) x vh (moving, 65 cols); both
                    # halves share one accumulation group: the first matmul's
                    # bank-wide has_written clear covers the whole acc tile
                    nc.tensor.matmul(
                        outT[n][:, 0:VW], et[:, sl:sl + 128],
                        vh[:, VW * gi:VW * gi + VW],
                        start=first, stop=False,
                    )
                    nc.tensor.matmul(
                        outT[n][:, VW:OW], et[:, sl + 128:sl + 256],
                        vh[:, VW * gi:VW * gi + VW],
                        start=False, stop=last,
                    )
                    pv_cnt[n] += 1
                    if pv_cnt[n] == T[n]:
                        acc = outT.pop(n)
                        slot = plan["order"].index(n)
                        nc.vector.tensor_copy(
                            out_sb[:, OW * slot:OW * (slot + 1)], acc[:]
                        )
                        done_slots[0] += 1
                        half, penult = nz // 2, max(nz - 1, nz // 2)
                        if done_slots[0] == half and half > 0:
                            nc.sync.dma_start(
                                out_d[:, 0:OW * half], out_sb[:, 0:OW * half]
                            )
                        elif done_slots[0] == penult and penult > half:
                            nc.sync.dma_start(
                                out_d[:, OW * half:OW * penult],
                                out_sb[:, OW * half:OW * penult],
                            )
                        elif done_slots[0] == nz:
                            lo = OW * penult
                            nc.sync.dma_start(
                                out_d[:, lo:OW * nz], out_sb[:, lo:OW * nz]
                            )

            for g in range(ngroups):
                part = seq[g * GROUP:(g + 1) * GROUP]
                st = stp.tile([128, 256 * GROUP], f32, tag="st")
                if g == 0:
                    for _ in range(NDUM):
                        nc.tensor.matmul(st[:64, 0:256], dums[:, :64], dums[:],
                                         start=True, stop=True)
                # scores; per bank exactly one accumulation group (start on
                # first write, stop on last)
                bank_writes = [0] * 3
                for i in range(len(part)):
                    bank_writes[SLOT[i] // 2] += 1
                bank_seen = [0] * 3
                for i, (n, t) in enumerate(part):
                    gi = int(goff[n]) + t
                    bank = SLOT[i] // 2
                    bank_seen[bank] += 1
                    sl = 256 * SLOT[i]
                    nc.tensor.matmul(
                        st[:, sl:sl + 256],
                        ktp[:, STS * gi:STS * (gi + 1)],
                        qt[:, QPC * n:QPC * (n + 1)],
                        start=(bank_seen[bank] == 1),
                        stop=(bank_seen[bank] == bank_writes[bank]),
                    )
                et = etp.tile([128, 256 * GROUP], f16, tag="et")
                # exp only the written slots (partial last group is sparse),
                # as contiguous runs, split at the bank 0-1 / bank 2 boundary
                slots = sorted(SLOT[i] for i in range(len(part)))
                runs = []
                for sl in slots:
                    if runs and runs[-1][1] == sl:
                        runs[-1][1] = sl + 1
                    else:
                        runs.append([sl, sl + 1])
                for lo_s, hi_s in runs:
                    lo, hi = 256 * lo_s, 256 * hi_s
                    if lo < 1024:
                        h = min(hi, 1024)
                        nc.scalar.activation(et[:, lo:h], st[:, lo:h], EXP,
                                             scale=0.125)
                    if hi > 1024:
                        l = max(lo, 1024)
                        nc.vector.tensor_scalar(
                            et[:, l:hi].bitcast(i16), st[:, l:hi],
                            SCH_A, SCH_B, MUL, ADD,
                        )
                pending.append((part, et))
                if len(pending) > 2:
                    _emit_pv(*pending.pop(0))

            while pending:
                _emit_pv(*pending.pop(0))

        nempty = sum(1 for x in T if x == 0)
        if nempty:
            lo = OW * (N - nempty)
            nc.vector.memset(out_sb[:, lo:OW * N], 0.0)
            nc.sync.dma_start(out_d[:, lo:OW * N], out_sb[:, lo:OW * N])

    nc.compile()
    return nc, plan


# --------------------------------------------------------------------------
# entry point
# --------------------------------------------------------------------------

def kernel(queries_nqd, keys_nsd, values_nsv, prefix_len_n):
    global LAST_RESULTS
    Q = np.ascontiguousarray(np.asarray(queries_nqd, dtype=np.float32))
    K = np.ascontiguousarray(np.asarray(keys_nsd, dtype=np.float32))
    V = np.ascontiguousarray(np.asarray(values_nsv, dtype=np.float32))
    p = [int(x) for x in np.asarray(prefix_len_n)]

    key = tuple(min(max(x, 0), S) for x in p)
    if key not in _program_cache:
        _program_cache[key] = _build_program(key)
    nc, plan = _program_cache[key]

    ktp, vh = _pack_shared(plan, K, V)
    in_maps = [dict(ktp=ktp, qt=_pack_core(plan, Q, c), vh=vh) for c in range(NCORES)]

    res = run_bass_kernel_spmd(nc, in_maps, list(range(NCORES)))
    LAST_RESULTS = res

    ocs = [np.asarray(res.results[c]["out"], dtype=np.float32) for c in range(NCORES)]
    return _gather(plan, ocs, Q, K, V)


def _gather(plan, ocs, Q, K, V):
    """Host-side gather: diagonal term + normalization (O(N*S*V) elementwise).
    Device A and Z carry an extra 1/8 (folded into vh); t matches it."""
    pa = np.asarray(plan["p"])
    t_nq = np.exp(np.einsum("nqd,nqd->nq", Q, K) * 0.125) * 0.125
    t_nq = np.where(np.arange(S)[None, :] >= pa[:, None], t_nq, 0.0).astype(np.float32)

    out = np.empty((N, S, VD), np.float32)
    for c, oc in enumerate(ocs):                               # oc [128, OW*N]
        for slot, n in enumerate(plan["order"]):
            if plan["T"][n] == 0:
                rows = slice(QPC * c, QPC * (c + 1))
                out[n, rows, :] = V[n, rows, :]
                continue
            for h in range(2):
                rows = slice(QPC * c + 128 * h, QPC * c + 128 * (h + 1))
                blk = oc[:, OW * slot + VW * h:OW * slot + VW * (h + 1)]
                A = blk[:, :VD]                                # [128, 64]
                Z = blk[:, VD]                                 # [128]
                t = t_nq[n, rows]
                out[n, rows, :] = (A + t[:, None] * V[n, rows, :]) / (Z + t)[:, None]
    return out
